# revision 8
# baseline (speedup 1.0000x reference)
"""MoE layer (24 experts, top-2 routing) on 8 Trainium2 NeuronCores.

Expert-parallel sharding: the host computes the gate routing (scores -> top-2
-> softmax combine weights), then dispatches each expert's tokens to the core
that owns the expert (3 experts per core, count-balanced by a sort-descending
assignment).  Each core runs one SPMD Bass/Tile program that, for each of its
3 expert slots, computes

    H^T[f, t] = gelu(w1^T-contract(x^T) + b1)      (MM1, K = d_model = 1024)
    Y^T[d, t] = w2^T-contract(H^T) + b2            (MM2, K = d_ff    = 4096)
    out       = Y^T * gate_weight[t]

with tokens on the matmul FREE dim, so per-expert token counts need no
128-padding (capacity = max count per slot across cores, rounded to even).
The host scatter-adds the per-expert outputs back into the [T, d] output
(the "combine" side of the all-to-all).

Matmuls run in bfloat16 (fp32 PSUM accumulate): the PE streams 1 output
column/cycle - the same rate as float32r - but weight DMA traffic halves
(the kernel is HBM-bound in fp32: ~100MB of weights per core per iteration
vs a ~220us PE floor).  Matmul relative error ~2.4e-3, well inside the 2e-2
gate.  Weight DMAs alternate between the two HWDGE rings (SP and ACT
issuing engines), which measured ~10% faster than a single ring.

Host-side work is routing/dispatch/combine only (index math, gather,
scatter-add); all FLOPs of the MoE layer itself (both matmuls, gelu, biases,
gate weighting) run on device.
"""

import sys

for _p in ("/opt/trn_rl_repo", "/root/.axon_site/_ro/trn_rl_repo"):
    if _p not in sys.path:
        sys.path.append(_p)

import numpy as np

import concourse.tile as tile
from concourse import bacc, mybir
from concourse.bass_utils import run_bass_kernel_spmd

B, S, D, FF, E, TOPK = 4, 1024, 1024, 4096, 24, 2
T = B * S
P = 128
KT1 = D // P     # 8  k-subtiles for MM1
MT1 = FF // P    # 32 f-tiles (MM1 output partition tiles)
KT2 = FF // P    # 32 k-subtiles for MM2
MT2 = D // P     # 8  d-tiles (MM2 output partition tiles)
N_CORES = 8
SLOTS = E // N_CORES  # 3 experts per core

F32R = mybir.dt.float32r
F32 = mybir.dt.float32
BF16 = mybir.dt.bfloat16

import ml_dtypes

NP_BF16 = ml_dtypes.bfloat16

_program_cache: dict = {}


def _build_program(caps, loop_reps=None, bench_internal_weights=False):
    """One SPMD program: SLOTS expert slots with token capacities caps[j].

    loop_reps: replicate the body N times (benchmark-only, to measure the
    steady-state device time via a wall-clock slope over N).
    bench_internal_weights: benchmark-only - weights live in internal DRAM
    scratch instead of ExternalInput so wall-clock timing excludes
    host->device shipping while keeping identical DMA traffic.
    """
    nc = bacc.Bacc("TRN2", target_bir_lowering=False, debug=False)

    wkind = "Internal" if bench_internal_weights else "ExternalInput"
    wsuff = "_int" if bench_internal_weights else ""
    w1t = nc.dram_tensor("w1t" + wsuff, (SLOTS, MT1, P, KT1, P), BF16, kind=wkind)
    w2t = nc.dram_tensor("w2t" + wsuff, (SLOTS, MT2, P, KT2, P), BF16, kind=wkind)
    b1t = nc.dram_tensor("b1t", (SLOTS, P, MT1), F32, kind="ExternalInput")
    b2t = nc.dram_tensor("b2t", (SLOTS, P, MT2), F32, kind="ExternalInput")
    xgs = [nc.dram_tensor(f"xg{j}", (P, KT1, caps[j]), BF16, kind="ExternalInput")
           for j in range(SLOTS)]
    gws = [nc.dram_tensor(f"gw{j}", (P, caps[j]), F32, kind="ExternalInput")
           for j in range(SLOTS)]
    ygs = [nc.dram_tensor(f"yg{j}", (MT2, P, caps[j]), F32, kind="ExternalOutput")
           for j in range(SLOTS)]

    with tile.TileContext(nc) as tc:
        with tc.tile_pool(name="xg", bufs=SLOTS + 1) as xg_pool, \
             tc.tile_pool(name="gw", bufs=SLOTS + 1) as gw_pool, \
             tc.tile_pool(name="bias", bufs=SLOTS + 1) as bias_pool, \
             tc.tile_pool(name="w1", bufs=4) as w1_pool, \
             tc.tile_pool(name="w2", bufs=3) as w2_pool, \
             tc.tile_pool(name="h", bufs=MT1) as h_pool, \
             tc.tile_pool(name="epi", bufs=4) as epi_pool, \
             tc.tile_pool(name="psa", bufs=4, space="PSUM") as psa, \
             tc.tile_pool(name="psb", bufs=4, space="PSUM") as psb:
            for _ in range(loop_reps or 1):
                dma_rr = [0]

                def wdma(dst, src):
                    # alternate weight DMAs across the two HWDGE rings
                    eng = nc.scalar if (dma_rr[0] % 2) else nc.sync
                    dma_rr[0] += 1
                    eng.dma_start(dst, src)

                # Preload every slot's activations/gates/biases so slot
                # boundaries never wait on small DMAs.
                slot_in = []
                for j in range(SLOTS):
                    C = caps[j]
                    xg_sb = xg_pool.tile([P, KT1, C], BF16, tag="xg")
                    nc.sync.dma_start(xg_sb[:], xgs[j].ap()[:])
                    gw_sb = gw_pool.tile([P, C], F32, tag="gw")
                    nc.sync.dma_start(gw_sb[:], gws[j].ap()[:])
                    b1_sb = bias_pool.tile([P, MT1], F32, tag="b1")
                    nc.sync.dma_start(b1_sb[:], b1t.ap()[j])
                    b2_sb = bias_pool.tile([P, MT2], F32, tag="b2")
                    nc.sync.dma_start(b2_sb[:], b2t.ap()[j])
                    slot_in.append((xg_sb, gw_sb, b1_sb, b2_sb))

                for j in range(SLOTS):
                    C = caps[j]
                    xg_sb, gw_sb, b1_sb, b2_sb = slot_in[j]

                    # Phase A: H^T tiles, one 128-row f-tile at a time.
                    h_tiles = []
                    for m in range(MT1):
                        w1_sb = w1_pool.tile([P, KT1, P], BF16, tag="w1")
                        wdma(w1_sb[:], w1t.ap()[j, m])
                        ph = psa.tile([P, C], F32, tag="psa")
                        for k in range(KT1):
                            nc.tensor.matmul(ph[:], w1_sb[:, k, :], xg_sb[:, k, :],
                                             start=(k == 0), stop=(k == KT1 - 1))
                        h_sb = h_pool.tile([P, C], BF16, tag="h")
                        nc.scalar.activation(h_sb[:], ph[:],
                                             mybir.ActivationFunctionType.Gelu,
                                             bias=b1_sb[:, m:m + 1])
                        h_tiles.append(h_sb)

                    # Phase B: Y^T tiles; epilogue adds b2, scales by gate.
                    for mo in range(MT2):
                        w2_sb = w2_pool.tile([P, KT2, P], BF16, tag="w2")
                        wdma(w2_sb[:], w2t.ap()[j, mo])
                        py = psb.tile([P, C], F32, tag="psb")
                        for k in range(KT2):
                            nc.tensor.matmul(py[:], w2_sb[:, k, :], h_tiles[k][:],
                                             start=(k == 0), stop=(k == KT2 - 1))
                        yb = epi_pool.tile([P, C], F32, tag="yb")
                        nc.scalar.activation(yb[:], py[:],
                                             mybir.ActivationFunctionType.Identity,
                                             bias=b2_sb[:, mo:mo + 1])
                        yo = epi_pool.tile([P, C], F32, tag="yo")
                        nc.vector.tensor_mul(yo[:], yb[:], gw_sb[:])
                        nc.sync.dma_start(ygs[j].ap()[mo], yo[:])
    nc.compile()
    return nc


def _route(x2d, gate_w, gate_b):
    """fp32 gate scores -> top-2 indices -> softmax combine weights."""
    scores = x2d @ gate_w + gate_b                               # [T, E]
    topi = np.argsort(-scores, axis=1, kind="stable")[:, :TOPK]  # [T, 2]
    topv = np.take_along_axis(scores, topi, axis=1)
    g = np.exp(topv - topv.max(axis=1, keepdims=True))
    g = g / g.sum(axis=1, keepdims=True)
    return topi, g.astype(np.float32)


def kernel(x, gate_w, gate_b, w1, b1, w2, b2):
    x = np.ascontiguousarray(np.asarray(x, dtype=np.float32))
    gate_w = np.asarray(gate_w, dtype=np.float32)
    gate_b = np.asarray(gate_b, dtype=np.float32)
    w1 = np.asarray(w1, dtype=np.float32)
    b1 = np.asarray(b1, dtype=np.float32)
    w2 = np.asarray(w2, dtype=np.float32)
    b2 = np.asarray(b2, dtype=np.float32)

    x2d = x.reshape(T, D)
    topi, gates = _route(x2d, gate_w, gate_b)

    # Token list and combine weight per expert (token order preserved).
    idx_e = [np.nonzero(topi == e)[0] for e in range(E)]
    gv_e = []
    for e in range(E):
        rows = topi == e                       # [T, 2] bool, <=1 True per row
        sel = rows.any(axis=1)
        gv_e.append(gates[sel, :][rows[sel, :]].astype(np.float32))
    counts = np.array([len(i) for i in idx_e])

    # Balance experts over (core, slot): sort by count descending; slot j
    # holds ranks [8j, 8j+8).  Slot capacity = max count in the slot,
    # rounded up to even (fp32r needs an even matmul free dim).
    order = np.argsort(-counts, kind="stable")
    slot_expert = np.empty((N_CORES, SLOTS), dtype=int)
    caps = []
    for j in range(SLOTS):
        ranks = order[j * N_CORES:(j + 1) * N_CORES]
        slot_expert[:, j] = ranks
        cmax = int(counts[ranks].max())
        caps.append(cmax + (cmax & 1))
    caps = tuple(caps)

    if caps not in _program_cache:
        _program_cache[caps] = _build_program(caps)
    nc = _program_cache[caps]

    xT = np.ascontiguousarray(x2d.T)                       # [D, T]
    in_maps = []
    for c in range(N_CORES):
        m = {}
        w1c = np.empty((SLOTS, MT1, P, KT1, P), NP_BF16)
        w2c = np.empty((SLOTS, MT2, P, KT2, P), NP_BF16)
        b1c = np.empty((SLOTS, P, MT1), np.float32)
        b2c = np.empty((SLOTS, P, MT2), np.float32)
        for j in range(SLOTS):
            e = int(slot_expert[c, j])
            C = caps[j]
            n = int(counts[e])
            xg = np.zeros((P, KT1, C), NP_BF16)
            xg[:, :, :n] = xT[:, idx_e[e]].reshape(KT1, P, n).transpose(1, 0, 2)
            m[f"xg{j}"] = xg
            gw = np.zeros((C,), np.float32)
            gw[:n] = gv_e[e]
            m[f"gw{j}"] = np.broadcast_to(gw, (P, C)).copy()
            # weight tiles in the exact SBUF layouts for single clean DMAs
            w1c[j] = w1[e].reshape(KT1, P, MT1, P).transpose(2, 1, 0, 3)
            w2c[j] = w2[e].reshape(KT2, P, MT2, P).transpose(2, 1, 0, 3)
            b1c[j] = b1[e].reshape(MT1, P).T
            b2c[j] = b2[e].reshape(MT2, P).T
        m["w1t"] = w1c
        m["w2t"] = w2c
        m["b1t"] = b1c
        m["b2t"] = b2c
        in_maps.append(m)

    res = run_bass_kernel_spmd(nc, in_maps, core_ids=list(range(N_CORES)))

    # Combine: scatter-add each expert's weighted outputs back to tokens.
    out = np.zeros((T, D), np.float32)
    for c in range(N_CORES):
        for j in range(SLOTS):
            e = int(slot_expert[c, j])
            n = int(counts[e])
            yg = res.results[c][f"yg{j}"].reshape(D, caps[j])
            out[idx_e[e], :] += yg[:, :n].T
    return out.reshape(B, S, D)



# revision 10
# speedup vs baseline: 2.8447x; 2.8447x over previous
"""MoE layer (24 experts, top-2 routing) on 8 Trainium2 NeuronCores.

Expert-parallel sharding: the host computes the gate routing (scores -> top-2
-> softmax combine weights), then dispatches each expert's tokens to the core
that owns the expert (3 experts per core, count-balanced by a sort-descending
assignment).  Each core runs one SPMD Bass/Tile program that, for each of its
3 expert slots, computes

    H^T[f, t] = gelu(w1^T-contract(x^T) + b1)      (MM1, K = d_model = 1024)
    Y^T[d, t] = w2^T-contract(H^T) + b2            (MM2, K = d_ff    = 4096)
    out       = Y^T * gate_weight[t]

with tokens on the matmul FREE dim, so per-expert token counts need no
128-padding (capacity = max count per slot across cores, rounded to even).
The host scatter-adds the per-expert outputs back into the [T, d] output
(the "combine" side of the all-to-all).

Matmuls run in bfloat16 (fp32 PSUM accumulate): the PE streams 1 output
column/cycle - the same rate as float32r - but weight DMA traffic halves
(the kernel is HBM-bound in fp32: ~100MB of weights per core per iteration
vs a ~220us PE floor).  Matmul relative error ~2.4e-3, well inside the 2e-2
gate.  Weight DMAs alternate between the two HWDGE rings (SP and ACT
issuing engines), which measured ~10% faster than a single ring.

Host-side work is routing/dispatch/combine only (index math, gather,
scatter-add); all FLOPs of the MoE layer itself (both matmuls, gelu, biases,
gate weighting) run on device.
"""

import sys

for _p in ("/opt/trn_rl_repo", "/root/.axon_site/_ro/trn_rl_repo"):
    if _p not in sys.path:
        sys.path.append(_p)

import numpy as np

import concourse.tile as tile
from concourse import bacc, mybir
from concourse.bass_utils import run_bass_kernel_spmd

B, S, D, FF, E, TOPK = 4, 1024, 1024, 4096, 24, 2
T = B * S
P = 128
KT1 = D // P     # 8  k-subtiles for MM1
MT1 = FF // P    # 32 f-tiles (MM1 output partition tiles)
KT2 = FF // P    # 32 k-subtiles for MM2
MT2 = D // P     # 8  d-tiles (MM2 output partition tiles)
N_CORES = 8
SLOTS = E // N_CORES  # 3 experts per core

F32R = mybir.dt.float32r
F32 = mybir.dt.float32
BF16 = mybir.dt.bfloat16

import ml_dtypes

NP_BF16 = ml_dtypes.bfloat16

_program_cache: dict = {}


def _build_program(caps, loop_reps=None, bench_internal_weights=False,
                   bench_internal_io=False):
    """One SPMD program: SLOTS expert slots with token capacities caps[j].

    loop_reps: run the body N times via a For_i hardware loop
    (benchmark-only, to measure the steady-state device time via a
    wall-clock slope over N - the loop body compiles once, so N can be
    large enough that on-device time dwarfs host/relay dispatch noise).
    bench_internal_weights: benchmark-only - weights live in internal DRAM
    scratch instead of ExternalInput so wall-clock timing excludes
    host->device shipping while keeping identical DMA traffic.
    bench_internal_io: benchmark-only - the yg outputs also live in
    internal DRAM scratch (same on-device DMA traffic, nothing shipped
    back); a tiny (P, MT2) dummy tensor is the only ExternalOutput.
    """
    nc = bacc.Bacc("TRN2", target_bir_lowering=False, debug=False)

    wkind = "Internal" if bench_internal_weights else "ExternalInput"
    wsuff = "_int" if bench_internal_weights else ""
    ykind = "Internal" if bench_internal_io else "ExternalOutput"
    ysuff = "_int" if bench_internal_io else ""
    w1t = nc.dram_tensor("w1t" + wsuff, (SLOTS, MT1, P, KT1, P), BF16, kind=wkind)
    w2t = nc.dram_tensor("w2t" + wsuff, (SLOTS, MT2, P, KT2, P), BF16, kind=wkind)
    b1t = nc.dram_tensor("b1t", (SLOTS, P, MT1), F32, kind="ExternalInput")
    b2t = nc.dram_tensor("b2t", (SLOTS, P, MT2), F32, kind="ExternalInput")
    xgs = [nc.dram_tensor(f"xg{j}", (P, KT1, caps[j]), BF16, kind="ExternalInput")
           for j in range(SLOTS)]
    gws = [nc.dram_tensor(f"gw{j}", (P, caps[j]), F32, kind="ExternalInput")
           for j in range(SLOTS)]
    ygs = [nc.dram_tensor(f"yg{j}" + ysuff, (MT2, P, caps[j]), F32, kind=ykind)
           for j in range(SLOTS)]
    dummy_out = (nc.dram_tensor("bench_out", (P, MT2), F32,
                                kind="ExternalOutput")
                 if bench_internal_io else None)

    import contextlib

    with tile.TileContext(nc) as tc:
        with tc.tile_pool(name="xg", bufs=SLOTS + 1) as xg_pool, \
             tc.tile_pool(name="gw", bufs=SLOTS + 1) as gw_pool, \
             tc.tile_pool(name="bias", bufs=SLOTS + 1) as bias_pool, \
             tc.tile_pool(name="w1", bufs=4) as w1_pool, \
             tc.tile_pool(name="w2", bufs=3) as w2_pool, \
             tc.tile_pool(name="h", bufs=MT1) as h_pool, \
             tc.tile_pool(name="epi", bufs=4) as epi_pool, \
             tc.tile_pool(name="dummy", bufs=1) as dummy_pool, \
             tc.tile_pool(name="psa", bufs=4, space="PSUM") as psa, \
             tc.tile_pool(name="psb", bufs=4, space="PSUM") as psb:
            loop_cm = (tc.For_i(0, loop_reps) if loop_reps and loop_reps > 1
                       else contextlib.nullcontext())
            with loop_cm:
                dma_rr = [0]

                def wdma(dst, src):
                    # alternate weight DMAs across the two HWDGE rings
                    eng = nc.scalar if (dma_rr[0] % 2) else nc.sync
                    dma_rr[0] += 1
                    eng.dma_start(dst, src)

                # Preload every slot's activations/gates/biases so slot
                # boundaries never wait on small DMAs.
                slot_in = []
                for j in range(SLOTS):
                    C = caps[j]
                    xg_sb = xg_pool.tile([P, KT1, C], BF16, tag="xg")
                    nc.sync.dma_start(xg_sb[:], xgs[j].ap()[:])
                    gw_sb = gw_pool.tile([P, C], F32, tag="gw")
                    nc.sync.dma_start(gw_sb[:], gws[j].ap()[:])
                    b1_sb = bias_pool.tile([P, MT1], F32, tag="b1")
                    nc.sync.dma_start(b1_sb[:], b1t.ap()[j])
                    b2_sb = bias_pool.tile([P, MT2], F32, tag="b2")
                    nc.sync.dma_start(b2_sb[:], b2t.ap()[j])
                    slot_in.append((xg_sb, gw_sb, b1_sb, b2_sb))

                for j in range(SLOTS):
                    C = caps[j]
                    xg_sb, gw_sb, b1_sb, b2_sb = slot_in[j]

                    # Phase A: H^T tiles, one 128-row f-tile at a time.
                    h_tiles = []
                    for m in range(MT1):
                        w1_sb = w1_pool.tile([P, KT1, P], BF16, tag="w1")
                        wdma(w1_sb[:], w1t.ap()[j, m])
                        ph = psa.tile([P, C], F32, tag="psa")
                        for k in range(KT1):
                            nc.tensor.matmul(ph[:], w1_sb[:, k, :], xg_sb[:, k, :],
                                             start=(k == 0), stop=(k == KT1 - 1))
                        h_sb = h_pool.tile([P, C], BF16, tag="h")
                        nc.scalar.activation(h_sb[:], ph[:],
                                             mybir.ActivationFunctionType.Gelu,
                                             bias=b1_sb[:, m:m + 1])
                        h_tiles.append(h_sb)

                    # Phase B: Y^T tiles; epilogue adds b2, scales by gate.
                    for mo in range(MT2):
                        w2_sb = w2_pool.tile([P, KT2, P], BF16, tag="w2")
                        wdma(w2_sb[:], w2t.ap()[j, mo])
                        py = psb.tile([P, C], F32, tag="psb")
                        for k in range(KT2):
                            nc.tensor.matmul(py[:], w2_sb[:, k, :], h_tiles[k][:],
                                             start=(k == 0), stop=(k == KT2 - 1))
                        yb = epi_pool.tile([P, C], F32, tag="yb")
                        nc.scalar.activation(yb[:], py[:],
                                             mybir.ActivationFunctionType.Identity,
                                             bias=b2_sb[:, mo:mo + 1])
                        yo = epi_pool.tile([P, C], F32, tag="yo")
                        nc.vector.tensor_mul(yo[:], yb[:], gw_sb[:])
                        nc.sync.dma_start(ygs[j].ap()[mo], yo[:])
            if dummy_out is not None:
                dsb = dummy_pool.tile([P, MT2], F32, tag="dummy")
                nc.sync.dma_start(dsb[:], b2t.ap()[0])
                nc.sync.dma_start(dummy_out.ap()[:], dsb[:])
    nc.compile()
    return nc


def _route(x2d, gate_w, gate_b):
    """fp32 gate scores -> top-2 indices -> softmax combine weights."""
    scores = x2d @ gate_w + gate_b                               # [T, E]
    topi = np.argsort(-scores, axis=1, kind="stable")[:, :TOPK]  # [T, 2]
    topv = np.take_along_axis(scores, topi, axis=1)
    g = np.exp(topv - topv.max(axis=1, keepdims=True))
    g = g / g.sum(axis=1, keepdims=True)
    return topi, g.astype(np.float32)


def kernel(x, gate_w, gate_b, w1, b1, w2, b2):
    x = np.ascontiguousarray(np.asarray(x, dtype=np.float32))
    gate_w = np.asarray(gate_w, dtype=np.float32)
    gate_b = np.asarray(gate_b, dtype=np.float32)
    w1 = np.asarray(w1, dtype=np.float32)
    b1 = np.asarray(b1, dtype=np.float32)
    w2 = np.asarray(w2, dtype=np.float32)
    b2 = np.asarray(b2, dtype=np.float32)

    x2d = x.reshape(T, D)
    topi, gates = _route(x2d, gate_w, gate_b)

    # Token list and combine weight per expert (token order preserved).
    idx_e = [np.nonzero(topi == e)[0] for e in range(E)]
    gv_e = []
    for e in range(E):
        rows = topi == e                       # [T, 2] bool, <=1 True per row
        sel = rows.any(axis=1)
        gv_e.append(gates[sel, :][rows[sel, :]].astype(np.float32))
    counts = np.array([len(i) for i in idx_e])

    # Balance experts over (core, slot): sort by count descending; slot j
    # holds ranks [8j, 8j+8).  Slot capacity = max count in the slot,
    # rounded up to even (fp32r needs an even matmul free dim).
    order = np.argsort(-counts, kind="stable")
    slot_expert = np.empty((N_CORES, SLOTS), dtype=int)
    caps = []
    for j in range(SLOTS):
        ranks = order[j * N_CORES:(j + 1) * N_CORES]
        slot_expert[:, j] = ranks
        cmax = int(counts[ranks].max())
        caps.append(cmax + (cmax & 1))
    caps = tuple(caps)

    if caps not in _program_cache:
        _program_cache[caps] = _build_program(caps)
    nc = _program_cache[caps]

    xT = np.ascontiguousarray(x2d.T)                       # [D, T]
    in_maps = []
    for c in range(N_CORES):
        m = {}
        w1c = np.empty((SLOTS, MT1, P, KT1, P), NP_BF16)
        w2c = np.empty((SLOTS, MT2, P, KT2, P), NP_BF16)
        b1c = np.empty((SLOTS, P, MT1), np.float32)
        b2c = np.empty((SLOTS, P, MT2), np.float32)
        for j in range(SLOTS):
            e = int(slot_expert[c, j])
            C = caps[j]
            n = int(counts[e])
            xg = np.zeros((P, KT1, C), NP_BF16)
            xg[:, :, :n] = xT[:, idx_e[e]].reshape(KT1, P, n).transpose(1, 0, 2)
            m[f"xg{j}"] = xg
            gw = np.zeros((C,), np.float32)
            gw[:n] = gv_e[e]
            m[f"gw{j}"] = np.broadcast_to(gw, (P, C)).copy()
            # weight tiles in the exact SBUF layouts for single clean DMAs
            w1c[j] = w1[e].reshape(KT1, P, MT1, P).transpose(2, 1, 0, 3)
            w2c[j] = w2[e].reshape(KT2, P, MT2, P).transpose(2, 1, 0, 3)
            b1c[j] = b1[e].reshape(MT1, P).T
            b2c[j] = b2[e].reshape(MT2, P).T
        m["w1t"] = w1c
        m["w2t"] = w2c
        m["b1t"] = b1c
        m["b2t"] = b2c
        in_maps.append(m)

    res = run_bass_kernel_spmd(nc, in_maps, core_ids=list(range(N_CORES)))

    # Combine: scatter-add each expert's weighted outputs back to tokens.
    out = np.zeros((T, D), np.float32)
    for c in range(N_CORES):
        for j in range(SLOTS):
            e = int(slot_expert[c, j])
            n = int(counts[e])
            yg = res.results[c][f"yg{j}"].reshape(D, caps[j])
            out[idx_e[e], :] += yg[:, :n].T
    return out.reshape(B, S, D)

